# revision 1
# baseline (speedup 1.0000x reference)
"""Trainium2 Bass kernel: fp8-quantized Dense (8192x4096 @ 4096x16384) + bias + tanh-GELU.

Strategy (tensor-parallel over units, 8 cores):
  - host: transpose x -> xT [d_in, tokens]; shard kernel/bias along units.
  - device per core:
      phase 1a: amax scan of this core's kernel shard (DVE abs-max), then
                AllReduce(max) -> global kernel amax (CC_k).
      phase 1b: amax scan of this core's 1/8 column slice of xT (overlaps
                CC_k), then AllReduce(max) -> global x amax (CC_x).
      phase 2:  quantize the kernel shard to fp8e4, resident in SBUF
                (first slabs are prefetched in f32 during the scans and
                quantized as soon as CC_k lands).
      phase 3:  stream xT token-blocks: quantize to fp8e4, DoubleRow fp8
                matmuls accumulating over d_in in PSUM, single-ACT epilogue
                gelu_tanh(psum * inv_scale + bias) per [128,512] tile, DMA out.
  - fp8 numerics: the reference quantizes with scale 448/amax onto the OCP
    e4m3fn grid. TRN fp8e4 tops out at 240 but matches e4m3fn exactly in
    [-240, 240]. Quantizing with 224/amax (= half the reference scale, a
    power-of-two ratio) lands on the identical grid after dequant: the hw
    fp8 values are exactly half the reference's. The dequant scale
    amax_x*amax_k/224^2 restores the reference computation up to f32
    accumulation order.
  - output is produced transposed ([units, tokens] per core); the host
    gathers shards and returns the [tokens, units] transpose view.
"""

import sys

sys.path.insert(0, "/opt/trn_rl_repo")

from contextlib import ExitStack

import numpy as np

import concourse.bacc as bacc
import concourse.tile as tile
from concourse import mybir
from concourse.bass_utils import run_bass_kernel_spmd

P = 128
FP8_HW_MAX = 224.0  # 448/2: keeps hw fp8 values inside TRN's +/-240 range

TOKENS, D_IN, UNITS, N_CORES = 8192, 4096, 16384, 8

KPRE_GROUPS = 2  # k-slab f32 prefetch groups staged in xq-pool slots
KPRE_PER_GROUP = 4


def _blocks(tokens, tblk):
    """Token-block schedule: small warmup blocks so PE starts earlier."""
    if tokens >= 4 * tblk and tblk >= 1024:
        small = tblk // 2
        rest = tokens - 4 * small
        assert rest % tblk == 0
        return [small] * 4 + [tblk] * (rest // tblk)
    assert tokens % tblk == 0
    return [tblk] * (tokens // tblk)


def build(tokens=TOKENS, d_in=D_IN, units=UNITS, n_cores=N_CORES, tblk=1024, nfree=512):
    us = units // n_cores
    ko_n = d_in // P          # 128-row f32 slabs along d_in
    kk_n = d_in // (2 * P)    # DoubleRow (256-contraction) steps
    nu = us // P              # 128-unit output blocks
    amx_t = tokens // n_cores # columns of xT this core amax-scans
    blocks = _blocks(tokens, tblk)

    assert d_in % (2 * P) == 0 and us % P == 0
    assert all(b % nfree == 0 for b in blocks)

    n_groups = min(KPRE_GROUPS, ko_n // KPRE_PER_GROUP)
    n_kpre = n_groups * KPRE_PER_GROUP

    dt = mybir.dt
    f32 = dt.float32
    fp8 = dt.float8e4
    X = mybir.AxisListType.X
    MAX = mybir.AluOpType.max

    nc = bacc.Bacc("TRN2", target_bir_lowering=False, debug=False, num_devices=n_cores)
    xT = nc.dram_tensor("xT", [d_in, tokens], f32, kind="ExternalInput").ap()
    xsl = nc.dram_tensor("xsl", [d_in, amx_t], f32, kind="ExternalInput").ap()
    ksh = nc.dram_tensor("ksh", [d_in, us], f32, kind="ExternalInput").ap()
    bsh = nc.dram_tensor("bsh", [us], f32, kind="ExternalInput").ap()
    out = nc.dram_tensor("out", [us, tokens], f32, kind="ExternalOutput").ap()

    smax = max(us, tblk, amx_t)

    def ldq(i):
        # alternate big loads across two HWDGE DMA queues (sync / scalar)
        return nc.sync if i % 2 == 0 else nc.scalar

    from concourse.tile_rust import add_dep_helper

    with tile.TileContext(nc) as tc, ExitStack() as ctx:
        const = ctx.enter_context(tc.tile_pool(name="const", bufs=1))
        kstage = ctx.enter_context(tc.tile_pool(name="kstage", bufs=3))
        xstage = ctx.enter_context(tc.tile_pool(name="xstage", bufs=5))
        kqp = ctx.enter_context(tc.tile_pool(name="kqp", bufs=1))
        xqp = ctx.enter_context(tc.tile_pool(name="xqp", bufs=2))
        outp = ctx.enter_context(tc.tile_pool(name="outp", bufs=8))
        psum = ctx.enter_context(tc.tile_pool(name="psum", bufs=8, space="PSUM"))
        dram = ctx.enter_context(tc.tile_pool(name="dram", bufs=1, space="DRAM"))
        small = ctx.enter_context(tc.tile_pool(name="small", bufs=1))
        xsmax = max(amx_t, tblk)

        from concourse import bass_isa

        def partition_amax_to(dst, racc, name):
            """[P, ko_n] per-partition maxes -> [1,1] scalar in dst (SBUF)."""
            col = small.tile([P, 1], f32, name=f"{name}_col")
            nc.vector.tensor_reduce(col[:], racc[:], axis=X, op=MAX)
            nc.gpsimd.partition_all_reduce(col[:], col[:], P, bass_isa.ReduceOp.max)
            nc.vector.tensor_copy(dst, col[0:1, :])

        def allreduce_max(src8, name):
            """AllReduce(max) of a [1,8] SBUF tile; returns [1,8] SBUF result."""
            cc_in = dram.tile([1, 8], f32, name=f"{name}_in")
            nc.sync.dma_start(cc_in[:], src8[:])
            cc_out = dram.tile([1, 8], f32, name=f"{name}_out", addr_space="Shared")
            nc.gpsimd.collective_compute(
                "AllReduce", MAX,
                replica_groups=[list(range(n_cores))],
                ins=[cc_in[:].opt()], outs=[cc_out[:].opt()],
            )
            res = small.tile([1, 8], f32, name=f"{name}_res")
            nc.sync.dma_start(res[:], cc_out[:])
            return res

        def bcast_scalar(src11, name):
            """[1,1] SBUF scalar (partition 0) -> [P,1] SBUF broadcast tile."""
            b = const.tile([P, 1], f32, name=f"{name}_b")
            nc.gpsimd.partition_broadcast(b[:], src11)
            return b

        # ---- phase 1a: kernel-shard amax scan (first on the wire) ----
        # The last n_kpre slabs land in the (still idle) xq-pool slots and are
        # RETAINED in f32 until the scale arrives, so they need no re-stream.
        rk_all = const.tile([P, ko_n], f32, name="rk_all")
        n_stream = ko_n - n_kpre
        last_kscan_dma = None
        for ko in range(n_stream):
            st = kstage.tile([P, us], f32, tag="kst", name="amx_k_st")
            last_kscan_dma = ldq(ko).dma_start(st[:], ksh[ko * P : (ko + 1) * P, :])
            nc.vector.tensor_reduce(
                rk_all[:, ko : ko + 1], st[:], axis=X, op=MAX,
                apply_absolute_value=True,
            )
        kret = []
        for g in range(n_groups):
            t = xqp.tile([P, KPRE_PER_GROUP, us], f32, tag="xq", name=f"kret{g}")
            for j in range(KPRE_PER_GROUP):
                ko = n_stream + g * KPRE_PER_GROUP + j
                last_kscan_dma = ldq(ko).dma_start(
                    t[:, j], ksh[ko * P : (ko + 1) * P, :]
                )
                nc.vector.tensor_reduce(
                    rk_all[:, ko : ko + 1], t[:, j], axis=X, op=MAX,
                    apply_absolute_value=True,
                )
            kret.append(t)

        # ---- phase 1b: x-slice amax scan (after the k-scan wire-wise) ----
        rx_all = const.tile([P, ko_n], f32, name="rx_all")
        last_xscan_dma = None
        for ko in range(ko_n):
            st = xstage.tile([P, xsmax], f32, tag="xst", name="amx_x_st")
            last_xscan_dma = ldq(ko).dma_start(
                st[:, :amx_t], xsl[ko * P : (ko + 1) * P, :]
            )
            if ko == 0 and last_kscan_dma is not None:
                add_dep_helper(
                    last_xscan_dma.ins, last_kscan_dma.ins, sync=True,
                    reason="x-scan starts after k-scan",
                )
            nc.vector.tensor_reduce(
                rx_all[:, ko : ko + 1], st[:, :amx_t], axis=X, op=MAX,
                apply_absolute_value=True,
            )

        # ---- single AllReduce(max) of [amax_k, amax_x] ----
        pk8 = small.tile([1, 8], f32, name="pk8")
        nc.vector.memset(pk8[:], 0.0)
        partition_amax_to(pk8[:, 0:1], rk_all, "pk")
        partition_amax_to(pk8[:, 1:2], rx_all, "px")
        g8 = allreduce_max(pk8, "cc")

        d2 = small.tile([1, 2], f32, name="d2")
        nc.vector.tensor_scalar_max(d2[:], g8[:, 0:2], 1e-12)

        # Correctly-rounded s = RNE(224/d): the quantize grid must bit-match the
        # reference's RNE(448/d)/2. DVE has no divide, and reciprocal+multiply
        # is 1-2 ulp off, which flips RNE decisions for ~1e-6 of elements and
        # costs ~2e-3 absmax error. Instead: Newton-refine 224*recip(d) with a
        # Dekker-exact residual (lands within ~0.51 ulp), then pick among 5
        # float-constructed neighbor candidates the one minimizing |q*d - 224|.
        # Verified in numpy over 300k random/binade-edge d with seeds up to
        # +-3 ulp off: 0 mismatches vs IEEE f32 division.
        NCAND = 5
        u32 = dt.uint32
        MUL = mybir.AluOpType.mult
        SUB = mybir.AluOpType.subtract
        ADD = mybir.AluOpType.add

        def tt(out, a, bb, op):
            nc.vector.tensor_tensor(out, a, bb, op)

        def c3(name):
            return small.tile([1, 2, NCAND], f32, name=name)

        def vsplit(src, pref, shape=(1, 2)):
            t_ = small.tile(list(shape), f32, name=f"{pref}_t")
            nc.vector.tensor_scalar_mul(t_[:], src, 4097.0)
            a_ = small.tile(list(shape), f32, name=f"{pref}_a")
            tt(a_[:], t_[:], src, SUB)
            hi = small.tile(list(shape), f32, name=f"{pref}_hi")
            tt(hi[:], t_[:], a_[:], SUB)
            lo = small.tile(list(shape), f32, name=f"{pref}_lo")
            tt(lo[:], src, hi[:], SUB)
            return hi, lo

        dh, dl = vsplit(d2[:], "dsp")

        def resid(qap, out_name, shape, dhb, dlb, db):
            """exact q*d - 224 via Dekker two-product (f32 ops only)"""
            p_ = small.tile(list(shape), f32, name=f"{out_name}_p")
            tt(p_[:], qap, db, MUL)
            qh, ql = vsplit(qap, f"{out_name}_qs", shape)
            w = small.tile(list(shape), f32, name=f"{out_name}_w")
            tt(w[:], qh[:], dhb, MUL)
            tt(w[:], w[:], p_[:], SUB)
            w2 = small.tile(list(shape), f32, name=f"{out_name}_w2")
            tt(w2[:], qh[:], dlb, MUL)
            tt(w[:], w[:], w2[:], ADD)
            tt(w2[:], ql[:], dhb, MUL)
            tt(w[:], w[:], w2[:], ADD)
            tt(w2[:], ql[:], dlb, MUL)
            tt(w[:], w[:], w2[:], ADD)
            nc.vector.tensor_scalar_sub(p_[:], p_[:], FP8_HW_MAX)
            R_ = small.tile(list(shape), f32, name=f"{out_name}_R")
            tt(R_[:], p_[:], w[:], ADD)
            return R_

        r2 = small.tile([1, 2], f32, name="r2")
        nc.vector.reciprocal(r2[:], d2[:])
        y0 = small.tile([1, 2], f32, name="y0")
        nc.vector.tensor_scalar_mul(y0[:], r2[:], FP8_HW_MAX)
        R0 = resid(y0[:], "n0", (1, 2), dh[:], dl[:], d2[:])
        corr = small.tile([1, 2], f32, name="corr")
        tt(corr[:], R0[:], r2[:], MUL)
        y = small.tile([1, 2], f32, name="yref")
        tt(y[:], y0[:], corr[:], SUB)

        # ulp(y) from the exponent bits; 5 candidates covering both binade sides
        um = small.tile([1, 2], f32, name="um")
        nc.vector.tensor_scalar(
            um[:].bitcast(u32), y[:].bitcast(u32), 0x7F800000, None,
            mybir.AluOpType.bitwise_and,
        )
        ul = small.tile([1, 2], f32, name="ul")
        nc.vector.tensor_scalar_mul(ul[:], um[:], 2.0 ** -23)
        cand = c3("cand")
        nc.vector.tensor_copy(cand[:, :, 0:1], y[:, :, None])
        tt(cand[:, :, 1:2], y[:, :, None], ul[:, :, None], ADD)
        tt(cand[:, :, 2:3], y[:, :, None], ul[:, :, None], SUB)
        nc.vector.tensor_scalar_mul(cand[:, :, 3:4], y[:, :, None], 1.0 - 2.0 ** -24)
        nc.vector.tensor_scalar_mul(cand[:, :, 4:5], y[:, :, None], 1.0 + 2.0 ** -24)

        dhb = dh[:, :, None].to_broadcast((1, 2, NCAND))
        dlb = dl[:, :, None].to_broadcast((1, 2, NCAND))
        db = d2[:, :, None].to_broadcast((1, 2, NCAND))
        Rc = resid(cand[:], "cc", (1, 2, NCAND), dhb, dlb, db)
        R2c = c3("R2c")
        tt(R2c[:], Rc[:], Rc[:], MUL)
        minr = small.tile([1, 2], f32, name="minr")
        nc.vector.tensor_reduce(minr[:], R2c[:], axis=X, op=mybir.AluOpType.min)
        mask = c3("mask")
        tt(mask[:], R2c[:], minr[:, :, None].to_broadcast((1, 2, NCAND)),
           mybir.AluOpType.is_equal)
        qm = c3("qm")
        tt(qm[:], cand[:], mask[:], MUL)
        s2 = small.tile([1, 2], f32, name="s2")
        nc.vector.tensor_reduce(s2[:], qm[:], axis=X, op=MAX)
        sk_b = bcast_scalar(s2[:, 0:1], "sk")
        sx_b = bcast_scalar(s2[:, 1:2], "sx")

        inv1 = small.tile([1, 1], f32, name="inv1")
        nc.vector.tensor_tensor(inv1[:], d2[:, 0:1], d2[:, 1:2], mybir.AluOpType.mult)
        nc.vector.tensor_scalar_mul(inv1[:], inv1[:], 1.0 / (FP8_HW_MAX * FP8_HW_MAX))
        inv_b = bcast_scalar(inv1[:], "inv")

        # bias shard, [P, nu]: bias_t[p, ub] = bias[ub*128 + p]
        bias_t = const.tile([P, nu], f32, name="bias_t")
        nc.sync.dma_start(bias_t[:], bsh.rearrange("(o p) -> p o", p=P))

        # ---- phase 2: quantize kernel shard, resident fp8 [P, ko_n, us] ----
        # Two-step quantize everywhere: an in-place f32 multiply (DVE, rounds
        # RNE32 exactly like the reference's x*scale) then a separate fp8
        # convert (ACT). The DVE's fused multiply+fp8-convert rounds the exact
        # product once, which disagrees with the reference's two roundings for
        # ~1e-6 of near-tie elements (~2e-3 absmax).
        kq = kqp.tile([P, ko_n, us], fp8, name="kq")
        for g in range(len(kret)):
            for j in range(KPRE_PER_GROUP):
                ko = n_stream + g * KPRE_PER_GROUP + j
                nc.vector.tensor_scalar_mul(kret[g][:, j], kret[g][:, j], sk_b[:])
                nc.scalar.copy(kq[:, ko], kret[g][:, j])
        last_restream_dma = last_xscan_dma
        for ko in range(n_stream):
            st = kstage.tile([P, us], f32, tag="kst", name="kq_st")
            dma = ldq(ko).dma_start(st[:], ksh[ko * P : (ko + 1) * P, :])
            if ko == 0 and last_xscan_dma is not None:
                add_dep_helper(
                    dma.ins, last_xscan_dma.ins, sync=True,
                    reason="kq re-stream starts after the scans",
                )
            last_restream_dma = dma
            nc.vector.tensor_scalar_mul(st[:], st[:], sk_b[:])
            nc.scalar.copy(kq[:, ko], st[:])

        # ---- phase 3: stream x blocks, fp8 DoubleRow matmuls, fused epilogue ----
        gelu = mybir.ActivationFunctionType.Gelu_apprx_tanh
        dr = mybir.MatmulPerfMode.DoubleRow
        t0 = 0
        for tb, blk in enumerate(blocks):
            tt_n = blk // nfree
            xq = xqp.tile([P, ko_n, blk], fp8, tag="xq", name="xq")
            for ko in range(ko_n):
                st = xstage.tile([P, xsmax], f32, tag="xst", name="xq_st")
                dma = nc.sync.dma_start(
                    st[:, :blk], xT[ko * P : (ko + 1) * P, t0 : t0 + blk]
                )
                if ko == 0 and last_restream_dma is not None:
                    # chain block streams so later blocks never jump the queue
                    add_dep_helper(
                        dma.ins, last_restream_dma.ins, sync=True,
                        reason="x blocks stream in consumption order",
                    )
                nc.vector.tensor_scalar_mul(st[:, :blk], st[:, :blk], sx_b[:])
                nc.any.tensor_copy(xq[:, ko], st[:, :blk])
            last_restream_dma = dma
            for ub in range(nu):
                pts = [
                    psum.tile([P, nfree], f32, tag="ps", name=f"ps{ti}")
                    for ti in range(tt_n)
                ]
                for kk in range(kk_n):
                    lw = kq[:, 2 * kk : 2 * kk + 2, ub * P : (ub + 1) * P]
                    for ti in range(tt_n):
                        nc.tensor.matmul(
                            pts[ti][:],
                            lw,
                            xq[:, 2 * kk : 2 * kk + 2, ti * nfree : (ti + 1) * nfree],
                            start=(kk == 0),
                            stop=(kk == kk_n - 1),
                            perf_mode=dr,
                        )
                for ti in range(tt_n):
                    ot = outp.tile([P, nfree], f32, tag="ot", name="ot")
                    nc.scalar.activation(
                        ot[:], pts[ti][:], gelu,
                        bias=bias_t[:, ub : ub + 1], scale=inv_b[:],
                    )
                    c0 = t0 + ti * nfree
                    nc.sync.dma_start(out[ub * P : (ub + 1) * P, c0 : c0 + nfree], ot[:])
            t0 += blk

    nc.compile()
    return nc


def make_in_maps(x, kern, bias, n_cores=N_CORES):
    tokens, d_in = x.shape
    us = kern.shape[1] // n_cores
    amx_t = tokens // n_cores
    xT = np.ascontiguousarray(x.T)
    in_maps = []
    for c in range(n_cores):
        in_maps.append(
            {
                "xT": xT,
                "xsl": np.ascontiguousarray(xT[:, c * amx_t : (c + 1) * amx_t]),
                "ksh": np.ascontiguousarray(kern[:, c * us : (c + 1) * us]),
                "bsh": np.ascontiguousarray(bias[c * us : (c + 1) * us]),
            }
        )
    return in_maps


_CACHE = {}


def _built():
    if "nc" not in _CACHE:
        _CACHE["nc"] = build()
    return _CACHE["nc"]


def run(x, kern, bias, trace=False, **kwargs):
    """Run on hardware; returns (full_output, BassKernelResults)."""
    nc = _built()
    in_maps = make_in_maps(x, kern, bias)
    res = run_bass_kernel_spmd(
        nc, in_maps, core_ids=list(range(N_CORES)), trace=trace, **kwargs
    )
    shards = [res.results[c]["out"] for c in range(N_CORES)]
    full = np.concatenate(shards, axis=0)  # [units, tokens]
    return full.T, res


def kernel(x, kernel, bias):
    out, _ = run(
        np.ascontiguousarray(x, dtype=np.float32),
        np.ascontiguousarray(kernel, dtype=np.float32),
        np.ascontiguousarray(bias, dtype=np.float32),
    )
    return out



# revision 13
# speedup vs baseline: 1.0576x; 1.0576x over previous
"""Trainium2 Bass kernel: fp8-quantized Dense (8192x4096 @ 4096x16384) + bias + tanh-GELU.

Strategy (tensor-parallel over units, 8 cores), v3 "fast prologue":
  - host: transpose x -> xT [d_in, tokens]; shard kernel/bias along units.
  - device per core:
      phase 1: amax scan of the kernel shard (4-queue DMA spread), issue
               AllReduce(max) for k IMMEDIATELY; then amax scan of this
               core's 1/8 column slice of xT, issue AllReduce(max) for x.
               The first 12 k-slabs are retained in SBUF f32 (8 in the
               idle xq-pool slots + 4 in a dedicated pool) so only 20
               slabs need re-streaming.
      phase 2: as soon as CC_k lands: compute the correctly-rounded scale
               sk, quantize retained slabs on the ACT engine (fused
               scale+fp8-convert), and re-stream the remaining slabs on
               the vector/tensor DMA queues (no head-of-line conflict
               with the x block loads on sync/scalar).
      phase 3: block 0 (512 tokens) runs kk-OUTER over two halves of 8
               psum banks so matmuls start as soon as CC_x lands and
               consume re-streamed k slabs as they arrive; later blocks
               run the usual ub-outer/kk-inner accumulation. x blocks are
               quantized with a single fused DVE multiply->fp8 op.
               Epilogue: one ACT op gelu_tanh(psum*inv_scale + bias) per
               [128,512] tile, written as fp16 and DMA'd out (halves the
               output traffic; host upcasts to f32).
  - fp8 numerics: the reference quantizes with scale 448/amax onto the OCP
    e4m3fn grid. TRN fp8e4 tops out at 240 but matches e4m3fn exactly in
    [-240, 240]. Quantizing with 224/amax (= half the reference scale, a
    power-of-two ratio) lands on the identical grid after dequant. The
    dequant scale amax_x*amax_k/224^2 restores the reference computation
    up to f32 accumulation order. The scale itself must be the correctly
    rounded f32 division RNE(224/amax) (a 1-ulp-off scale shifts the
    whole grid: measured 2.4e-3 rel err), hence the Newton+candidate
    selection sequence below.
  - output is produced transposed ([units, tokens] per core) in fp16
    (4.3e-4 rel err, well under the 2e-2 gate); the host gathers shards
    and returns the [tokens, units] f32 view.
"""

import sys

sys.path.insert(0, "/opt/trn_rl_repo")

from contextlib import ExitStack

import numpy as np

import concourse.bacc as bacc
import concourse.tile as tile
from concourse import mybir
from concourse.bass_utils import run_bass_kernel_spmd

P = 128
FP8_HW_MAX = 224.0  # 448/2: keeps hw fp8 values inside TRN's +/-240 range

TOKENS, D_IN, UNITS, N_CORES = 8192, 4096, 16384, 8

RET_GROUPS = 2  # retained k-slab groups of 4 (first 8 slabs = kk 0..3)
RET_PER_GROUP = 4
B0 = 256  # first token block: fully f32-staged so its loads never stall


def _blocks(tokens, tblk):
    """Token-block schedule: small warmup blocks so PE starts earlier."""
    assert tokens >= 2 * tblk and tblk >= 1024
    head = [B0, B0, 2 * B0]
    rest = tokens - sum(head)
    assert rest % tblk == 0
    return head + [tblk] * (rest // tblk)


def build(tokens=TOKENS, d_in=D_IN, units=UNITS, n_cores=N_CORES, tblk=1024, nfree=512):
    us = units // n_cores
    ko_n = d_in // P          # 128-row f32 slabs along d_in
    kk_n = d_in // (2 * P)    # DoubleRow (256-contraction) steps
    nu = us // P              # 128-unit output blocks
    amx_t = tokens // n_cores # columns of xT this core amax-scans
    blocks = _blocks(tokens, tblk)

    assert d_in % (2 * P) == 0 and us % P == 0
    assert all(b % nfree == 0 or b in (B0, 2 * B0) for b in blocks)
    assert blocks[0] == B0 and nu == 16

    n_ret = RET_GROUPS * RET_PER_GROUP
    n_stream = ko_n - n_ret

    dt = mybir.dt
    f32 = dt.float32
    f16 = dt.float16
    fp8 = dt.float8e4
    X = mybir.AxisListType.X
    MAX = mybir.AluOpType.max
    COPY = mybir.ActivationFunctionType.Copy

    nc = bacc.Bacc("TRN2", target_bir_lowering=False, debug=False, num_devices=n_cores)
    xT = nc.dram_tensor("xT", [d_in, tokens], f32, kind="ExternalInput").ap()
    xsl = nc.dram_tensor("xsl", [d_in, amx_t], f32, kind="ExternalInput").ap()
    ksh = nc.dram_tensor("ksh", [d_in, us], f32, kind="ExternalInput").ap()
    bsh = nc.dram_tensor("bsh", [us], f32, kind="ExternalInput").ap()
    out = nc.dram_tensor("out", [us, tokens], f16, kind="ExternalOutput").ap()

    def scan_q(i):
        # only SP(sync) and Activation(scalar) have HWDGE queues
        return (nc.sync, nc.scalar)[i % 2]

    restream_q = scan_q
    xload_q = scan_q

    with tile.TileContext(nc) as tc, ExitStack() as ctx:
        const = ctx.enter_context(tc.tile_pool(name="const", bufs=1))
        kstage = ctx.enter_context(tc.tile_pool(name="kstage", bufs=3))
        xstage = ctx.enter_context(tc.tile_pool(name="xstage", bufs=4))
        kqp = ctx.enter_context(tc.tile_pool(name="kqp", bufs=1))
        xqp = ctx.enter_context(tc.tile_pool(name="xqp", bufs=2))
        xb0p = ctx.enter_context(tc.tile_pool(name="xb0p", bufs=1))
        outp = ctx.enter_context(tc.tile_pool(name="outp", bufs=5))
        psum = ctx.enter_context(tc.tile_pool(name="psum", bufs=8, space="PSUM"))
        dram = ctx.enter_context(tc.tile_pool(name="dram", bufs=1, space="DRAM"))
        small = ctx.enter_context(tc.tile_pool(name="small", bufs=1))

        from concourse import bass_isa

        def partition_amax_to(dst, racc, name):
            """[P, ko_n] per-partition maxes -> [1,1] scalar in dst (SBUF)."""
            col = small.tile([P, 1], f32, name=f"{name}_col")
            nc.vector.tensor_reduce(col[:], racc[:], axis=X, op=MAX)
            nc.gpsimd.partition_all_reduce(col[:], col[:], P, bass_isa.ReduceOp.max)
            nc.vector.tensor_copy(dst, col[0:1, :])

        def allreduce_max_issue(src11, name):
            """Issue AllReduce(max) of a [1,1] scalar; returns the shared
            dram tile holding the result (read back separately)."""
            cc_in = dram.tile([1, 8], f32, name=f"{name}_in")
            z8 = small.tile([1, 8], f32, name=f"{name}_z8")
            nc.vector.memset(z8[:], 0.0)
            nc.vector.tensor_copy(z8[:, 0:1], src11)
            nc.sync.dma_start(cc_in[:], z8[:])
            cc_out = dram.tile([1, 8], f32, name=f"{name}_out", addr_space="Shared")
            nc.gpsimd.collective_compute(
                "AllReduce", MAX,
                replica_groups=[list(range(n_cores))],
                ins=[cc_in[:].opt()], outs=[cc_out[:].opt()],
            )
            return cc_out

        def bcast_scalar(src11, name):
            """[1,1] SBUF scalar (partition 0) -> [P,1] SBUF broadcast tile."""
            b = const.tile([P, 1], f32, name=f"{name}_b")
            nc.gpsimd.partition_broadcast(b[:], src11)
            return b

        # Correctly-rounded s = RNE(224/d): the quantize grid must bit-match
        # the reference's RNE(448/d)/2. DVE has no divide, and a 1-2 ulp-off
        # scale shifts the whole fp8 grid (~2.4e-3 rel err). Newton-refine
        # 224*recip(d) with a Dekker-exact residual, then pick among 5
        # float-constructed neighbor candidates the one minimizing |q*d-224|.
        NCAND = 5
        u32 = dt.uint32
        MUL = mybir.AluOpType.mult
        SUB = mybir.AluOpType.subtract
        ADD = mybir.AluOpType.add

        def tt(o, a, bb, op):
            nc.vector.tensor_tensor(o, a, bb, op)

        def exact_scale(g8, name):
            """g8: [1,8] SBUF allreduce result (slot 0 = amax).
            Returns ([1,1] scale s = RNE(224/max(amax,1e-12)), [1,1] d)."""
            d1 = small.tile([1, 1], f32, name=f"{name}_d1")
            nc.vector.tensor_scalar_max(d1[:], g8[:, 0:1], 1e-12)

            def c3(nm):
                return small.tile([1, 1, NCAND], f32, name=f"{name}_{nm}")

            def vsplit(src, pref, shape=(1, 1)):
                t_ = small.tile(list(shape), f32, name=f"{name}_{pref}_t")
                nc.vector.tensor_scalar_mul(t_[:], src, 4097.0)
                a_ = small.tile(list(shape), f32, name=f"{name}_{pref}_a")
                tt(a_[:], t_[:], src, SUB)
                hi = small.tile(list(shape), f32, name=f"{name}_{pref}_hi")
                tt(hi[:], t_[:], a_[:], SUB)
                lo = small.tile(list(shape), f32, name=f"{name}_{pref}_lo")
                tt(lo[:], src, hi[:], SUB)
                return hi, lo

            dh, dl = vsplit(d1[:], "dsp")

            def resid(qap, nm, shape, dhb, dlb, db):
                """exact q*d - 224 via Dekker two-product (f32 ops only)"""
                p_ = small.tile(list(shape), f32, name=f"{name}_{nm}_p")
                tt(p_[:], qap, db, MUL)
                qh, ql = vsplit(qap, f"{nm}_qs", shape)
                w = small.tile(list(shape), f32, name=f"{name}_{nm}_w")
                tt(w[:], qh[:], dhb, MUL)
                tt(w[:], w[:], p_[:], SUB)
                w2 = small.tile(list(shape), f32, name=f"{name}_{nm}_w2")
                tt(w2[:], qh[:], dlb, MUL)
                tt(w[:], w[:], w2[:], ADD)
                tt(w2[:], ql[:], dhb, MUL)
                tt(w[:], w[:], w2[:], ADD)
                tt(w2[:], ql[:], dlb, MUL)
                tt(w[:], w[:], w2[:], ADD)
                nc.vector.tensor_scalar_sub(p_[:], p_[:], FP8_HW_MAX)
                R_ = small.tile(list(shape), f32, name=f"{name}_{nm}_R")
                tt(R_[:], p_[:], w[:], ADD)
                return R_

            r1 = small.tile([1, 1], f32, name=f"{name}_r1")
            nc.vector.reciprocal(r1[:], d1[:])
            y0 = small.tile([1, 1], f32, name=f"{name}_y0")
            nc.vector.tensor_scalar_mul(y0[:], r1[:], FP8_HW_MAX)
            R0 = resid(y0[:], "n0", (1, 1), dh[:], dl[:], d1[:])
            corr = small.tile([1, 1], f32, name=f"{name}_corr")
            tt(corr[:], R0[:], r1[:], MUL)
            y = small.tile([1, 1], f32, name=f"{name}_y")
            tt(y[:], y0[:], corr[:], SUB)

            um = small.tile([1, 1], f32, name=f"{name}_um")
            nc.vector.tensor_scalar(
                um[:].bitcast(u32), y[:].bitcast(u32), 0x7F800000, None,
                mybir.AluOpType.bitwise_and,
            )
            ul = small.tile([1, 1], f32, name=f"{name}_ul")
            nc.vector.tensor_scalar_mul(ul[:], um[:], 2.0 ** -23)
            cand = c3("cand")
            nc.vector.tensor_copy(cand[:, :, 0:1], y[:, :, None])
            tt(cand[:, :, 1:2], y[:, :, None], ul[:, :, None], ADD)
            tt(cand[:, :, 2:3], y[:, :, None], ul[:, :, None], SUB)
            nc.vector.tensor_scalar_mul(cand[:, :, 3:4], y[:, :, None], 1.0 - 2.0 ** -24)
            nc.vector.tensor_scalar_mul(cand[:, :, 4:5], y[:, :, None], 1.0 + 2.0 ** -24)

            dhb = dh[:, :, None].to_broadcast((1, 1, NCAND))
            dlb = dl[:, :, None].to_broadcast((1, 1, NCAND))
            db = d1[:, :, None].to_broadcast((1, 1, NCAND))
            Rc = resid(cand[:], "cc", (1, 1, NCAND), dhb, dlb, db)
            R2c = c3("R2c")
            tt(R2c[:], Rc[:], Rc[:], MUL)
            minr = small.tile([1, 1], f32, name=f"{name}_minr")
            nc.vector.tensor_reduce(minr[:], R2c[:], axis=X, op=mybir.AluOpType.min)
            mask = c3("mask")
            tt(mask[:], R2c[:], minr[:, :, None].to_broadcast((1, 1, NCAND)),
               mybir.AluOpType.is_equal)
            qm = c3("qm")
            tt(qm[:], cand[:], mask[:], MUL)
            s1 = small.tile([1, 1], f32, name=f"{name}_s1")
            nc.vector.tensor_reduce(s1[:], qm[:], axis=X, op=MAX)
            return s1, d1

        # ---- phase 1a: kernel-shard amax scan (first on the wire) ----
        # First 8 slabs (kk 0..3) are retained f32 in the idle xq-pool
        # slots. Remaining 24 stream through kstage.
        rk_all = const.tile([P, ko_n], f32, name="rk_all")
        kret = []
        for g in range(RET_GROUPS):
            t = xqp.tile([P, RET_PER_GROUP, us], f32, tag="xq", name=f"kret{g}")
            for j in range(RET_PER_GROUP):
                ko = g * RET_PER_GROUP + j
                scan_q(ko).dma_start(t[:, j], ksh[ko * P : (ko + 1) * P, :])
                nc.vector.tensor_reduce(
                    rk_all[:, ko : ko + 1], t[:, j], axis=X, op=MAX,
                    apply_absolute_value=True,
                )
            kret.append(t)
        for ko in range(n_ret, ko_n):
            st = kstage.tile([P, us], f32, tag="kst", name="amx_k_st")
            scan_q(ko).dma_start(st[:], ksh[ko * P : (ko + 1) * P, :])
            nc.vector.tensor_reduce(
                rk_all[:, ko : ko + 1], st[:], axis=X, op=MAX,
                apply_absolute_value=True,
            )

        # issue AllReduce(max) for k right away (overlaps the x scan)
        pk1 = small.tile([1, 1], f32, name="pk1")
        partition_amax_to(pk1[:], rk_all, "pk")
        cck_out = allreduce_max_issue(pk1[:], "cck")

        # ---- phase 1b: x-slice amax scan ----
        rx_all = const.tile([P, ko_n], f32, name="rx_all")
        for ko in range(ko_n):
            st = xstage.tile([P, tblk], f32, tag="xst", name="amx_x_st")
            scan_q(ko).dma_start(st[:, :amx_t], xsl[ko * P : (ko + 1) * P, :])
            nc.vector.tensor_reduce(
                rx_all[:, ko : ko + 1], st[:, :amx_t], axis=X, op=MAX,
                apply_absolute_value=True,
            )
        px1 = small.tile([1, 1], f32, name="px1")
        partition_amax_to(px1[:], rx_all, "px")
        ccx_out = allreduce_max_issue(px1[:], "ccx")

        # ---- k scale: read CC_k result (gpsimd queue; must not block the
        # sync/scalar/vector/tensor queues), exact division, broadcast ----
        gk = small.tile([1, 8], f32, name="gk")
        nc.gpsimd.dma_start(gk[:], cck_out[:])
        sk, dk = exact_scale(gk, "sk")
        sk_b = bcast_scalar(sk[:], "sk")

        # bias shard, [P, nu]: bias_t[p, ub] = bias[ub*128 + p]
        bias_t = const.tile([P, nu], f32, name="bias_t")
        nc.gpsimd.dma_start(bias_t[:], bsh.rearrange("(o p) -> p o", p=P))

        # ---- pre-stage block 0 (fully, in a dedicated f32 tile): these DMA
        # triggers never WAR-stall, so they fire as soon as the scan queues
        # drain and the data is resident long before CC_x lands. ----
        xb0 = xb0p.tile([P, ko_n, B0], f32, name="xb0")
        for ko in range(ko_n):
            scan_q(ko).dma_start(xb0[:, ko], xT[ko * P : (ko + 1) * P, 0:B0])

        # ---- phase 2: quantize kernel shard, resident fp8 [P, ko_n, us] ----
        # ACT engine does the fused scale-multiply + fp8 convert (one op per
        # slab) so the DVE stays free for the x-block quantizes. The grid is
        # identical to the reference's up to <=1e-6 of near-tie elements
        # (measured 8.5e-7 rel err on the real data). The retained-slab
        # quantizes are interleaved between the restream slabs so they fill
        # the ACT idle gaps while the restream DMAs are in flight.
        kq = kqp.tile([P, ko_n, us], fp8, name="kq")
        ret_jobs = [(g, j) for g in range(RET_GROUPS) for j in range(RET_PER_GROUP)]
        for i, ko in enumerate(range(n_ret, ko_n)):
            st = kstage.tile([P, us], f32, tag="kst", name="kq_st")
            restream_q(ko).dma_start(st[:], ksh[ko * P : (ko + 1) * P, :])
            nc.scalar.activation(kq[:, ko], st[:], COPY, scale=sk_b[:])
            if i < len(ret_jobs):
                g, j = ret_jobs[i]
                rko = g * RET_PER_GROUP + j
                nc.scalar.activation(kq[:, rko], kret[g][:, j], COPY, scale=sk_b[:])

        # ---- x scale (lands ~40us after the x scan ends) ----
        gx = small.tile([1, 8], f32, name="gx")
        nc.gpsimd.dma_start(gx[:], ccx_out[:])
        sx, dx = exact_scale(gx, "sx")
        sx_b = bcast_scalar(sx[:], "sx")

        inv1 = small.tile([1, 1], f32, name="inv1")
        nc.vector.tensor_tensor(inv1[:], dk[:], dx[:], MUL)
        nc.vector.tensor_scalar_mul(inv1[:], inv1[:], 1.0 / (FP8_HW_MAX * FP8_HW_MAX))
        inv_b = bcast_scalar(inv1[:], "inv")

        # ---- phase 3: stream x blocks, fp8 DoubleRow matmuls, fused epilogue ----
        gelu = mybir.ActivationFunctionType.Gelu_apprx_tanh
        dr = mybir.MatmulPerfMode.DoubleRow

        def load_and_quant_block(t0, blk, qi):
            """Stream one token block and DVE-fuse quantize into an xq tile."""
            xq = xqp.tile([P, ko_n, blk], fp8, tag="xq", name="xq")
            for ko in range(ko_n):
                st = xstage.tile([P, tblk], f32, tag="xst", name="xq_st")
                xload_q(qi + ko).dma_start(
                    st[:, :blk], xT[ko * P : (ko + 1) * P, t0 : t0 + blk]
                )
                nc.vector.tensor_scalar_mul(xq[:, ko], st[:, :blk], sx_b[:])
            return xq

        def epilogue(pt, ub, c0, w, qi):
            ot = outp.tile([P, nfree], f16, tag="ot", name="ot")
            nc.scalar.activation(
                ot[:, :w], pt[:, :w], gelu,
                bias=bias_t[:, ub : ub + 1], scale=inv_b[:],
            )
            xload_q(qi).dma_start(out[ub * P : (ub + 1) * P, c0 : c0 + w], ot[:, :w])

        t0 = 0
        for tb, blk in enumerate(blocks):
            if tb == 0:
                # Block 0: quantize the pre-staged f32 tile (no DMA on the
                # critical path once CC_x lands), then kk-OUTER over two
                # halves of 8 psum banks so the accumulations advance at
                # the pace the re-streamed k slabs arrive -- the PE starts
                # ~40us before kq is complete.
                xq = xqp.tile([P, ko_n, blk], fp8, tag="xq", name="xq0")
                for ko in range(ko_n):
                    nc.vector.tensor_scalar_mul(xq[:, ko], xb0[:, ko], sx_b[:])
                for half in range(2):
                    u0 = half * 8
                    pts = [
                        psum.tile([P, nfree], f32, tag="ps", name=f"ps0_{u0+i}")
                        for i in range(8)
                    ]
                    for kk in range(kk_n):
                        for i in range(8):
                            ub = u0 + i
                            nc.tensor.matmul(
                                pts[i][:, :blk],
                                kq[:, 2 * kk : 2 * kk + 2, ub * P : (ub + 1) * P],
                                xq[:, 2 * kk : 2 * kk + 2, :],
                                start=(kk == 0),
                                stop=(kk == kk_n - 1),
                                perf_mode=dr,
                            )
                    for i in range(8):
                        epilogue(pts[i], u0 + i, t0, blk, u0 + i)
            else:
                tt_n = max(1, blk // nfree)
                w = min(blk, nfree)
                xq = load_and_quant_block(t0, blk, tb)
                for ub in range(nu):
                    pts = [
                        psum.tile([P, nfree], f32, tag="ps", name=f"ps{ti}")
                        for ti in range(tt_n)
                    ]
                    for kk in range(kk_n):
                        lw = kq[:, 2 * kk : 2 * kk + 2, ub * P : (ub + 1) * P]
                        for ti in range(tt_n):
                            nc.tensor.matmul(
                                pts[ti][:, :w],
                                lw,
                                xq[:, 2 * kk : 2 * kk + 2, ti * w : (ti + 1) * w],
                                start=(kk == 0),
                                stop=(kk == kk_n - 1),
                                perf_mode=dr,
                            )
                    for ti in range(tt_n):
                        epilogue(pts[ti], ub, t0 + ti * w, w, ub + ti)
            t0 += blk

    nc.compile()
    return nc


def make_in_maps(x, kern, bias, n_cores=N_CORES):
    tokens, d_in = x.shape
    us = kern.shape[1] // n_cores
    amx_t = tokens // n_cores
    xT = np.ascontiguousarray(x.T)
    in_maps = []
    for c in range(n_cores):
        in_maps.append(
            {
                "xT": xT,
                "xsl": np.ascontiguousarray(xT[:, c * amx_t : (c + 1) * amx_t]),
                "ksh": np.ascontiguousarray(kern[:, c * us : (c + 1) * us]),
                "bsh": np.ascontiguousarray(bias[c * us : (c + 1) * us]),
            }
        )
    return in_maps


_CACHE = {}


def _built():
    if "nc" not in _CACHE:
        _CACHE["nc"] = build()
    return _CACHE["nc"]


def run(x, kern, bias, trace=False, **kwargs):
    """Run on hardware; returns (full_output, BassKernelResults)."""
    nc = _built()
    in_maps = make_in_maps(x, kern, bias)
    res = run_bass_kernel_spmd(
        nc, in_maps, core_ids=list(range(N_CORES)), trace=trace, **kwargs
    )
    shards = [res.results[c]["out"] for c in range(N_CORES)]
    full = np.concatenate(shards, axis=0)  # [units, tokens] fp16
    return full.T.astype(np.float32), res


def kernel(x, kernel, bias):
    out, _ = run(
        np.ascontiguousarray(x, dtype=np.float32),
        np.ascontiguousarray(kernel, dtype=np.float32),
        np.ascontiguousarray(bias, dtype=np.float32),
    )
    return out
